# revision 12
# baseline (speedup 1.0000x reference)
"""Block-causal (block=64) MHA + qkv/out projections on 8 NeuronCores.

Sharding: 8 cores = 2 batches x 4 head-groups (4 heads each).
Per core: qkv projection for its heads, block-causal attention for 4 heads
(processed as 2 head-pairs packed across the 128 partitions), partial output
projection over its 256 channels. Host sums the 4 partials per batch + bias.

On-chip layout is feature-major (transposed): scores are computed transposed
(S^T[k, q] = k . q) so no on-chip transposes are needed anywhere; softmax
denominators (sums over the key/partition axis) come from an all-ones matmul
on the PE, broadcast across 64 partitions. exp runs on ScalarE straight out
of PSUM.

Matmuls run in float32r (1 PE cycle/row vs 4 for f32). f32r imposes two
constraints honored throughout: every matmul-operand tile is f32r-typed
(producers round on write; memset can't, so constants stage through f32 +
tensor_copy), and matmul outputs must start at PSUM partition 0 (tile_position
col 0) - so each head accumulates PV/sum in partitions 0:64 of its own bank,
attention outputs live in per-head [64, 512] tiles, and the out-projection
chains 4 K=64 matmuls (one per head).

The diagonal 128-key tiles are one full matmul over queries [q0:512) whose
disallowed corner (keys 64:128 x queries [q0:q0+64)) is zeroed post-exp, so
block-causality needs no masking ops in the main rectangles.
"""

import numpy as np

import concourse.bass as bass
import concourse.tile as tile
from concourse import bacc
from concourse import mybir

B, N, C = 2, 2048, 1024
H, HD = 16, 64
HPC = 4  # heads per core
CSL = HPC * HD  # 256 channel slice per core
QKW = 2 * CSL  # 512: q then k output channels
NCORES = 8
QBLK = 512
NQB = N // QBLK  # 4
NT = N // 128  # 16 seq tiles of 128
SCALE = HD**-0.5
F32 = mybir.dt.float32
F32R = mybir.dt.float32r
MMDT = F32R  # matmul-operand dtype: f32r runs the PE at 1 cycle/row


def build_nc():
    nc = bacc.Bacc("TRN2", target_bir_lowering=False, debug=False, num_devices=NCORES)

    xT_d = nc.dram_tensor("xT", [8, 128, N], MMDT, kind="ExternalInput")
    wqk_d = nc.dram_tensor("wqkT", [8, 128, QKW], MMDT, kind="ExternalInput")
    wv_d = nc.dram_tensor("wvT", [8, 128, CSL], MMDT, kind="ExternalInput")
    wp_d = nc.dram_tensor("wpT", [4, 64, C], MMDT, kind="ExternalInput")
    y_d = nc.dram_tensor("y", [N, C], F32, kind="ExternalOutput")

    with tile.TileContext(nc) as tc:
        with (
            tc.tile_pool(name="persist", bufs=1) as persist,
            tc.tile_pool(name="pt", bufs=2) as pt_pool,
            tc.tile_pool(name="rc", bufs=1) as rc_pool,
            tc.tile_pool(name="att", bufs=1) as at_pool,
            tc.tile_pool(name="yout", bufs=3) as y_pool,
            tc.tile_pool(name="psmm", bufs=2, space="PSUM") as ps_mm,
            tc.tile_pool(name="pssc", bufs=1, space="PSUM") as ps_sc,
            tc.tile_pool(name="psacc", bufs=1, space="PSUM") as ps_acc,
        ):
            # ---- load inputs (one tile per DMA so consumers wait on few sems) ----
            xts = [persist.tile([128, N], MMDT, tag=f"xt{i}", name=f"xt{i}") for i in range(8)]
            wqks = [persist.tile([128, QKW], MMDT, tag=f"wqk{i}", name=f"wqk{i}") for i in range(8)]
            wvs = [persist.tile([128, CSL], MMDT, tag=f"wv{i}", name=f"wv{i}") for i in range(8)]
            wp4 = [persist.tile([64, C], MMDT, tag=f"wp{i}", name=f"wp{i}") for i in range(4)]
            for ct in range(8):
                nc.sync.dma_start(out=xts[ct], in_=xT_d[ct])
                nc.sync.dma_start(out=wqks[ct], in_=wqk_d[ct])
                nc.sync.dma_start(out=wvs[ct], in_=wv_d[ct])
            for h in range(4):
                nc.sync.dma_start(out=wp4[h], in_=wp_d[h])

            # memset can't write f32r (ISA check); stage via f32 + rounding copy
            ones_t = persist.tile([128, 128], MMDT, tag="ones")
            ones_f = persist.tile([128, 128], F32, tag="ones_f")
            nc.vector.memset(ones_f, 1.0)
            nc.vector.tensor_copy(out=ones_t, in_=ones_f)
            zero_r = persist.tile([128, 64], MMDT, tag="zero_r")
            zero_f = persist.tile([128, 64], F32, tag="zero_f")
            nc.vector.memset(zero_f, 0.0)
            nc.vector.tensor_copy(out=zero_r, in_=zero_f)

            # persistent diagonal p tiles, one per (j, head-half). Their
            # disallowed corner (keys 64:128 x queries [128j, 128j+64)) is
            # zeroed ONCE here; the diag exps write around it, so PV/sum can
            # read the full [*, q0:512) range with the corner always zero.
            pd = {}
            for j in range(4):
                for hh in range(2):
                    t = persist.tile(
                        [128, QBLK], MMDT, tag=f"pd{j}{hh}", name=f"pd{j}{hh}"
                    )
                    pd[j, hh] = t
                    nc.gpsimd.tensor_copy(
                        out=t[64:128, 128 * j : 128 * j + 64], in_=zero_r[64:128, :]
                    )

            # ---- phase 1: q/k projection, transposed outputs ----
            # qkT tiles: 0 = q heads(0,1), 1 = q heads(2,3), 2 = k(0,1), 3 = k(2,3)
            # within a tile: partitions 0:64 = first head dims, 64:128 = second.
            qkT = [persist.tile([128, N], MMDT, tag=f"qk{t}", name=f"qk{t}") for t in range(4)]
            for dt_ in range(4):
                for nb in range(NQB):
                    ps = ps_mm.tile([128, QBLK], F32, tag="mm")
                    for ct in range(8):
                        nc.tensor.matmul(
                            ps,
                            lhsT=wqks[ct][:, dt_ * 128 : (dt_ + 1) * 128],
                            rhs=xts[ct][:, nb * QBLK : (nb + 1) * QBLK],
                            start=(ct == 0),
                            stop=(ct == 7),
                        )
                    nc.vector.tensor_copy(
                        out=qkT[dt_][:, nb * QBLK : (nb + 1) * QBLK], in_=ps
                    )

            # ---- phase 2: v projection, natural layout [n, 4*64] ----
            v_sb = [persist.tile([128, CSL], MMDT, tag=f"v{t}", name=f"v{t}") for t in range(NT)]
            for nt in range(NT):
                ps = ps_mm.tile([128, CSL], F32, tag="mm")
                for ct in range(8):
                    nc.tensor.matmul(
                        ps,
                        lhsT=xts[ct][:, nt * 128 : (nt + 1) * 128],
                        rhs=wvs[ct],
                        start=(ct == 0),
                        stop=(ct == 7),
                    )
                nc.vector.tensor_copy(out=v_sb[nt], in_=ps)

            # ---- phase 3+4: attention (per 512-query block), then out-proj ----
            for qi in range(NQB):
                # per-head attention outputs for this query block, all at
                # partition base 0 (f32r matmuls can't write PSUM base 64)
                a4 = [
                    at_pool.tile([64, QBLK], MMDT, tag=f"a{h}", name=f"a{h}_{qi}")
                    for h in range(4)
                ]
                for pair in range(2):
                    qt = qkT[pair]
                    kt_t = qkT[2 + pair]
                    qs = slice(qi * QBLK, (qi + 1) * QBLK)

                    # one PSUM bank per head per accumulator; heads use
                    # partitions 0:64 only (f32r base-0 rule).
                    at_bA = ps_acc.tile([128, QBLK], F32, tag="atA", name="at_bA")
                    at_bB = ps_acc.tile([128, QBLK], F32, tag="atB", name="at_bB")
                    sm_bA = ps_acc.tile([128, QBLK], F32, tag="smA", name="sm_bA")
                    sm_bB = ps_acc.tile([128, QBLK], F32, tag="smB", name="sm_bB")

                    n_reg = 4 * qi
                    n_per_range = n_reg + 4
                    at_A, at_B, sm_A, sm_B = [0], [0], [0], [0]

                    def fl(cnt, total=n_per_range):
                        i = cnt[0]
                        cnt[0] += 1
                        return dict(start=(i == 0), stop=(i == total - 1))

                    vA_of = lambda kt: v_sb[kt][:, pair * 128 : pair * 128 + 64]
                    vB_of = lambda kt: v_sb[kt][:, pair * 128 + 64 : pair * 128 + 128]

                    # fully-causal key tiles: whole [128k x 512q] rectangles
                    for kt in range(n_reg):
                        ks = slice(kt * 128, (kt + 1) * 128)
                        psA = ps_sc.tile([128, QBLK], F32, tag="sA")
                        psB = ps_sc.tile([128, QBLK], F32, tag="sB")
                        nc.tensor.matmul(
                            psA, lhsT=kt_t[0:64, ks], rhs=qt[0:64, qs],
                            start=True, stop=True,
                        )
                        nc.tensor.matmul(
                            psB, lhsT=kt_t[64:128, ks], rhs=qt[64:128, qs],
                            start=True, stop=True,
                        )
                        pA = pt_pool.tile([128, QBLK], MMDT, tag="pA")
                        pB = pt_pool.tile([128, QBLK], MMDT, tag="pB")
                        nc.scalar.activation(
                            out=pA, in_=psA, func=mybir.ActivationFunctionType.Exp,
                            scale=SCALE,
                        )
                        nc.scalar.activation(
                            out=pB, in_=psB, func=mybir.ActivationFunctionType.Exp,
                            scale=SCALE,
                        )
                        nc.tensor.matmul(
                            at_bA[0:64, :], lhsT=vA_of(kt), rhs=pA, **fl(at_A)
                        )
                        nc.tensor.matmul(
                            at_bB[0:64, :], lhsT=vB_of(kt), rhs=pB, **fl(at_B)
                        )
                        # full-width ones stationary: a 128x128 tile runs
                        # ~1.6x faster per row than a 64-col one, and the
                        # replicated sums land on every lane
                        nc.tensor.matmul(
                            sm_bA, lhsT=ones_t, rhs=pA, **fl(sm_A)
                        )
                        nc.tensor.matmul(
                            sm_bB, lhsT=ones_t, rhs=pB, **fl(sm_B)
                        )

                    # diagonal key tiles: one full 128-key matmul over queries
                    # [q0:512); the disallowed corner (keys 64:128 x queries
                    # [q0:q0+64)) is zeroed after the exp so PV/sum can run as
                    # single K=128 matmuls.
                    for j in range(4):
                        kt = 4 * qi + j
                        q0 = 128 * j
                        q1 = q0 + 64
                        ksl = slice(kt * 128, (kt + 1) * 128)
                        qsl = slice(qi * QBLK + q0, (qi + 1) * QBLK)
                        psA = ps_sc.tile([128, QBLK], F32, tag="sA")
                        psB = ps_sc.tile([128, QBLK], F32, tag="sB")
                        pA = pd[j, 0]
                        pB = pd[j, 1]
                        for ph, ps_s, p_s in ((0, psA, pA), (64, psB, pB)):
                            hd_sl = slice(ph, ph + 64)
                            nc.tensor.matmul(
                                ps_s[:, q0:QBLK], lhsT=kt_t[hd_sl, ksl],
                                rhs=qt[hd_sl, qsl], start=True, stop=True,
                            )
                            # two exps that skirt the pre-zeroed corner
                            # (keys 64:128 x queries [q0:q1))
                            nc.scalar.activation(
                                out=p_s[0:64, q0:QBLK], in_=ps_s[0:64, q0:QBLK],
                                func=mybir.ActivationFunctionType.Exp, scale=SCALE,
                            )
                            nc.scalar.activation(
                                out=p_s[64:128, q1:QBLK], in_=ps_s[64:128, q1:QBLK],
                                func=mybir.ActivationFunctionType.Exp, scale=SCALE,
                            )
                        for p_s, at_c, sm_c, at_b, sm_b, v_of in (
                            (pA, at_A, sm_A, at_bA, sm_bA, vA_of),
                            (pB, at_B, sm_B, at_bB, sm_bB, vB_of),
                        ):
                            nc.tensor.matmul(
                                at_b[0:64, q0:QBLK], lhsT=v_of(kt),
                                rhs=p_s[:, q0:QBLK], **fl(at_c),
                            )
                            nc.tensor.matmul(
                                sm_b[:, q0:QBLK], lhsT=ones_t,
                                rhs=p_s[:, q0:QBLK], **fl(sm_c),
                            )

                    # normalize: a4[2*pair+h] = at * (1 / sm), per head
                    for h, at_b, sm_b in ((0, at_bA, sm_bA), (1, at_bB, sm_bB)):
                        rec = rc_pool.tile([64, QBLK], F32, tag=f"rc{h}")
                        nc.vector.reciprocal_approx_fast(out=rec, in_=sm_b[0:64, :])
                        nc.vector.tensor_mul(
                            out=a4[2 * pair + h], in0=at_b[0:64, :], in1=rec
                        )

                # output projection for this query block's 4 row tiles:
                # 4 chained K=64 matmuls (one per head), all at base 0
                for nt in range(4 * qi, 4 * qi + 4):
                    ysb = y_pool.tile([128, C], F32, tag="y")
                    ntl = (nt - 4 * qi) * 128
                    for cb in range(2):
                        psy = ps_mm.tile([128, QBLK], F32, tag="mm")
                        for h in range(4):
                            nc.tensor.matmul(
                                psy,
                                lhsT=a4[h][:, ntl : ntl + 128],
                                rhs=wp4[h][:, cb * QBLK : (cb + 1) * QBLK],
                                start=(h == 0),
                                stop=(h == 3),
                            )
                        nc.vector.tensor_copy(
                            out=ysb[:, cb * QBLK : (cb + 1) * QBLK], in_=psy
                        )
                    nc.sync.dma_start(out=y_d[nt * 128 : (nt + 1) * 128, :], in_=ysb)

    return nc


def _shard_inputs(x, w_qkv, w_proj):
    x = np.ascontiguousarray(np.asarray(x, dtype=np.float32))
    w_qkv = np.asarray(w_qkv, dtype=np.float32)
    w_proj = np.asarray(w_proj, dtype=np.float32)
    xT = [np.ascontiguousarray(x[b].T).reshape(8, 128, N) for b in range(B)]
    in_maps = []
    for c in range(NCORES):
        b, g = divmod(c, 4)
        r0 = 64 * HPC * g  # 256 * g
        wq = w_qkv[r0 : r0 + CSL, :]
        wk = w_qkv[C + r0 : C + r0 + CSL, :]
        wvs = w_qkv[2 * C + r0 : 2 * C + r0 + CSL, :]
        wqkT = np.ascontiguousarray(np.concatenate([wq, wk], axis=0).T)
        wvT = np.ascontiguousarray(wvs.T)
        wpT = np.ascontiguousarray(w_proj[:, r0 : r0 + CSL].T)
        in_maps.append(
            {
                "xT": xT[b],
                "wqkT": wqkT.reshape(8, 128, QKW),
                "wvT": wvT.reshape(8, 128, CSL),
                "wpT": wpT.reshape(4, 64, C),
            }
        )
    return in_maps


def run(x, w_qkv, w_proj, b_proj, trace=False, **spmd_kwargs):
    from concourse.bass_utils import run_bass_kernel_spmd

    in_maps = _shard_inputs(x, w_qkv, w_proj)
    nc = build_nc()
    nc.finalize()
    res = run_bass_kernel_spmd(
        nc, in_maps, core_ids=list(range(NCORES)), trace=trace, **spmd_kwargs
    )
    y = np.zeros((B, N, C), np.float32)
    for c in range(NCORES):
        y[c // 4] += res.results[c]["y"]
    y += np.asarray(b_proj, dtype=np.float32)[None, None, :]
    return y, res


def kernel(x, w_qkv, w_proj, b_proj):
    y, _ = run(x, w_qkv, w_proj, b_proj, trace=False)
    return y


# revision 15
# speedup vs baseline: 1.2923x; 1.2923x over previous
"""Block-causal (block=64) MHA + qkv/out projections on 8 NeuronCores.

Sharding: 8 cores = 2 batches x 4 head-groups (4 heads each).
Per core: qkv projection for its heads, block-causal attention for 4 heads
(processed as 2 head-pairs packed across the 128 partitions), partial output
projection over its 256 channels. Host sums the 4 partials per batch + bias.

On-chip layout is feature-major (transposed): scores are computed transposed
(S^T[k, q] = k . q) so no on-chip transposes are needed anywhere. exp runs on
ScalarE straight out of PSUM.

Matmuls run in float32r (full PE rate). f32r rules honored throughout: every
matmul-operand tile is f32r-typed (producers round on write; memset can't
write f32r, so constants stage through f32 + tensor_copy), and matmul outputs
must start at PSUM partition 0.

The softmax denominator rides in the PV matmul for free: the V tiles are laid
out [v0 | 1 | v1 | v2 | 1 | v3] (384 cols) so each head h has a contiguous
128-col [v_h | ones] / [ones | v_h] stationary window; one full 128x128
matmul per (key tile, head) then yields PV rows on one partition half and the
replicated key-sums on the other. Normalization: reciprocal_approx_fast reads
the sum rows from PSUM, a small SBUF->SBUF DMA shifts the reciprocals onto
the PV lanes, and two tensor_muls write the pair-packed [128, 512] attention
tile, which feeds a 2-chain K=128 output projection.

The diagonal 128-key tiles are one matmul over queries [q0:512) into
persistent per-(j, half) p tiles whose disallowed corner (keys 64:128 x
queries [q0:q0+64)) was zeroed once at startup; the exps write around the
corner, so block-causality costs no per-iteration masking.
"""

import numpy as np

import concourse.bass as bass
import concourse.tile as tile
from concourse import bacc
from concourse import mybir

B, N, C = 2, 2048, 1024
H, HD = 16, 64
HPC = 4  # heads per core
CSL = HPC * HD  # 256 channel slice per core
QKW = 2 * CSL  # 512: q then k output channels
NCORES = 8
QBLK = 512
NQB = N // QBLK  # 4
NT = N // 128  # 16 seq tiles of 128
SCALE = HD**-0.5
F32 = mybir.dt.float32
F32R = mybir.dt.float32r
MMDT = F32R  # matmul-operand dtype: f32r runs the PE at 1 cycle/row


def build_nc():
    nc = bacc.Bacc("TRN2", target_bir_lowering=False, debug=False, num_devices=NCORES)

    xT_d = nc.dram_tensor("xT", [8, 128, N], MMDT, kind="ExternalInput")
    wqk_d = nc.dram_tensor("wqkT", [8, 128, QKW], MMDT, kind="ExternalInput")
    wv_d = nc.dram_tensor("wvT", [8, 128, CSL], MMDT, kind="ExternalInput")
    wp_d = nc.dram_tensor("wpT", [2, 128, C], MMDT, kind="ExternalInput")
    y_d = nc.dram_tensor("y", [N, C], F32, kind="ExternalOutput")

    with tile.TileContext(nc) as tc:
        with (
            tc.tile_pool(name="persist", bufs=1) as persist,
            tc.tile_pool(name="pt", bufs=2) as pt_pool,
            tc.tile_pool(name="rc", bufs=2) as rc_pool,
            tc.tile_pool(name="att", bufs=2) as at_pool,
            tc.tile_pool(name="yout", bufs=2) as y_pool,
            tc.tile_pool(name="psmm", bufs=2, space="PSUM") as ps_mm,
            tc.tile_pool(name="pssc", bufs=2, space="PSUM") as ps_sc,
            tc.tile_pool(name="psacc", bufs=1, space="PSUM") as ps_acc,
        ):
            # ---- load inputs (one tile per DMA so consumers wait on few sems) ----
            xts = [persist.tile([128, N], MMDT, tag=f"xt{i}", name=f"xt{i}") for i in range(8)]
            wqks = [persist.tile([128, QKW], MMDT, tag=f"wqk{i}", name=f"wqk{i}") for i in range(8)]
            wvs = [persist.tile([128, CSL], MMDT, tag=f"wv{i}", name=f"wv{i}") for i in range(8)]
            wp2 = [persist.tile([128, C], MMDT, tag=f"wp{i}", name=f"wp{i}") for i in range(2)]
            for ct in range(8):
                nc.sync.dma_start(out=xts[ct], in_=xT_d[ct])
                nc.sync.dma_start(out=wqks[ct], in_=wqk_d[ct])
                nc.sync.dma_start(out=wvs[ct], in_=wv_d[ct])
            for pr in range(2):
                nc.sync.dma_start(out=wp2[pr], in_=wp_d[pr])

            # memset can't write f32r (ISA check); stage via f32 + rounding copy
            ones_t = persist.tile([128, 128], MMDT, tag="ones")
            ones_f = persist.tile([128, 128], F32, tag="ones_f")
            nc.vector.memset(ones_f, 1.0)
            nc.vector.tensor_copy(out=ones_t, in_=ones_f)
            zero_r = persist.tile([128, 64], MMDT, tag="zero_r")
            zero_f = persist.tile([128, 64], F32, tag="zero_f")
            nc.vector.memset(zero_f, 0.0)
            nc.vector.tensor_copy(out=zero_r, in_=zero_f)

            # persistent diagonal p tiles, one per (j, head-half). Their
            # disallowed corner (keys 64:128 x queries [128j, 128j+64)) is
            # zeroed ONCE here; the diag exps write around it, so PV/sum can
            # read the full [*, q0:512) range with the corner always zero.
            pd = {}
            for j in range(4):
                for hh in range(2):
                    t = persist.tile(
                        [128, QBLK], MMDT, tag=f"pd{j}{hh}", name=f"pd{j}{hh}"
                    )
                    pd[j, hh] = t
                    nc.gpsimd.tensor_copy(
                        out=t[64:128, 128 * j : 128 * j + 64], in_=zero_r[64:128, :]
                    )

            # ---- phase 1: q/k projection, transposed outputs ----
            # qkT tiles: 0 = q heads(0,1), 1 = q heads(2,3), 2 = k(0,1), 3 = k(2,3)
            # within a tile: partitions 0:64 = first head dims, 64:128 = second.
            qkT = [persist.tile([128, N], MMDT, tag=f"qk{t}", name=f"qk{t}") for t in range(4)]
            for dt_ in range(4):
                for nb in range(NQB):
                    ps = ps_mm.tile([128, QBLK], F32, tag="mm")
                    for ct in range(8):
                        nc.tensor.matmul(
                            ps,
                            lhsT=wqks[ct][:, dt_ * 128 : (dt_ + 1) * 128],
                            rhs=xts[ct][:, nb * QBLK : (nb + 1) * QBLK],
                            start=(ct == 0),
                            stop=(ct == 7),
                        )
                    nc.vector.tensor_copy(
                        out=qkT[dt_][:, nb * QBLK : (nb + 1) * QBLK], in_=ps
                    )

            # ---- phase 2: v projection into [v0 | 1 | v1 | v2 | 1 | v3] ----
            # head h's PV stationary is the 128-col window starting at 64*h
            # offset... concretely: head 0 -> cols 0:128 ([v0|1]), head 1 ->
            # cols 64:192 ([1|v1]), head 2 -> 192:320 ([v2|1]), head 3 ->
            # 256:384 ([1|v3]).
            v65 = [persist.tile([128, 384], MMDT, tag=f"v{t}", name=f"v{t}") for t in range(NT)]
            for nt in range(NT):
                nc.gpsimd.tensor_copy(out=v65[nt][:, 64:128], in_=ones_t[:, 0:64])
                nc.gpsimd.tensor_copy(out=v65[nt][:, 256:320], in_=ones_t[:, 0:64])
                ps = ps_mm.tile([128, CSL], F32, tag="mm")
                for ct in range(8):
                    nc.tensor.matmul(
                        ps,
                        lhsT=xts[ct][:, nt * 128 : (nt + 1) * 128],
                        rhs=wvs[ct],
                        start=(ct == 0),
                        stop=(ct == 7),
                    )
                nc.vector.tensor_copy(out=v65[nt][:, 0:64], in_=ps[:, 0:64])
                nc.vector.tensor_copy(out=v65[nt][:, 128:256], in_=ps[:, 64:192])
                nc.vector.tensor_copy(out=v65[nt][:, 320:384], in_=ps[:, 192:256])

            # stationary windows: (even head A, odd head B) per pair
            def vwin(kt, pair, hh):
                base = pair * 192 + hh * 64
                return v65[kt][:, base : base + 128]

            # ---- phase 3+4: attention (per 512-query block), then out-proj ----
            for qi in range(NQB):
                a2 = [
                    at_pool.tile([128, QBLK], MMDT, tag=f"a{p}", name=f"a{p}_{qi}")
                    for p in range(2)
                ]
                for pair in range(2):
                    qt = qkT[pair]
                    kt_t = qkT[2 + pair]
                    qs = slice(qi * QBLK, (qi + 1) * QBLK)

                    # one PSUM bank per head: PV rows on one partition half,
                    # replicated key-sums on the other (from the ones block
                    # of the stationary). A: PV 0:64 / sums 64:128; B: sums
                    # 0:64 / PV 64:128.
                    at_bA = ps_acc.tile([128, QBLK], F32, tag="atA", name="at_bA")
                    at_bB = ps_acc.tile([128, QBLK], F32, tag="atB", name="at_bB")

                    n_reg = 4 * qi
                    n_per_range = n_reg + 4
                    at_A, at_B = [0], [0]

                    def fl(cnt, total=n_per_range):
                        i = cnt[0]
                        cnt[0] += 1
                        return dict(start=(i == 0), stop=(i == total - 1))

                    # fully-causal key tiles: whole [128k x 512q] rectangles
                    for kt in range(n_reg):
                        ks = slice(kt * 128, (kt + 1) * 128)
                        psA = ps_sc.tile([128, QBLK], F32, tag="sA")
                        psB = ps_sc.tile([128, QBLK], F32, tag="sB")
                        nc.tensor.matmul(
                            psA, lhsT=kt_t[0:64, ks], rhs=qt[0:64, qs],
                            start=True, stop=True,
                        )
                        nc.tensor.matmul(
                            psB, lhsT=kt_t[64:128, ks], rhs=qt[64:128, qs],
                            start=True, stop=True,
                        )
                        pA = pt_pool.tile([128, QBLK], MMDT, tag="pA")
                        pB = pt_pool.tile([128, QBLK], MMDT, tag="pB")
                        nc.scalar.activation(
                            out=pA, in_=psA, func=mybir.ActivationFunctionType.Exp,
                            scale=SCALE,
                        )
                        nc.scalar.activation(
                            out=pB, in_=psB, func=mybir.ActivationFunctionType.Exp,
                            scale=SCALE,
                        )
                        nc.tensor.matmul(
                            at_bA, lhsT=vwin(kt, pair, 0), rhs=pA, **fl(at_A)
                        )
                        nc.tensor.matmul(
                            at_bB, lhsT=vwin(kt, pair, 1), rhs=pB, **fl(at_B)
                        )

                    # diagonal key tiles: one matmul over queries [q0:512);
                    # exps skirt the pre-zeroed corner
                    for j in range(4):
                        kt = 4 * qi + j
                        q0 = 128 * j
                        q1 = q0 + 64
                        ksl = slice(kt * 128, (kt + 1) * 128)
                        qsl = slice(qi * QBLK + q0, (qi + 1) * QBLK)
                        psA = ps_sc.tile([128, QBLK], F32, tag="sA")
                        psB = ps_sc.tile([128, QBLK], F32, tag="sB")
                        pA = pd[j, 0]
                        pB = pd[j, 1]
                        for ph, ps_s, p_s in ((0, psA, pA), (64, psB, pB)):
                            hd_sl = slice(ph, ph + 64)
                            nc.tensor.matmul(
                                ps_s[:, q0:QBLK], lhsT=kt_t[hd_sl, ksl],
                                rhs=qt[hd_sl, qsl], start=True, stop=True,
                            )
                            nc.scalar.activation(
                                out=p_s[0:64, q0:QBLK], in_=ps_s[0:64, q0:QBLK],
                                func=mybir.ActivationFunctionType.Exp, scale=SCALE,
                            )
                            nc.scalar.activation(
                                out=p_s[64:128, q1:QBLK], in_=ps_s[64:128, q1:QBLK],
                                func=mybir.ActivationFunctionType.Exp, scale=SCALE,
                            )
                        nc.tensor.matmul(
                            at_bA[:, q0:QBLK], lhsT=vwin(kt, pair, 0),
                            rhs=pA[:, q0:QBLK], **fl(at_A),
                        )
                        nc.tensor.matmul(
                            at_bB[:, q0:QBLK], lhsT=vwin(kt, pair, 1),
                            rhs=pB[:, q0:QBLK], **fl(at_B),
                        )

                    # normalize: reciprocal straight out of PSUM (doubles as
                    # the PSUM->SBUF copy), DMA shifts recips onto PV lanes,
                    # two muls write the pair-packed attention tile
                    rec = rc_pool.tile([128, QBLK], F32, tag="rec")
                    rsh = rc_pool.tile([128, QBLK], F32, tag="rsh")
                    # A's sums sit at PSUM base 64: the custom-DVE approx
                    # reciprocal misreads PSUM there (observed on HW), so
                    # that half uses the exact (slower) reciprocal, which
                    # reads PSUM base-64 correctly.
                    nc.vector.reciprocal(
                        out=rec[64:128, :], in_=at_bA[64:128, :]
                    )
                    nc.vector.reciprocal_approx_fast(
                        out=rec[0:64, :], in_=at_bB[0:64, :]
                    )
                    nc.sync.dma_start(out=rsh[0:64, :], in_=rec[64:128, :])
                    nc.sync.dma_start(out=rsh[64:128, :], in_=rec[0:64, :])
                    nc.vector.tensor_mul(
                        out=a2[pair][0:64, :], in0=at_bA[0:64, :], in1=rsh[0:64, :]
                    )
                    nc.vector.tensor_mul(
                        out=a2[pair][64:128, :], in0=at_bB[64:128, :],
                        in1=rsh[64:128, :],
                    )

                # output projection for this query block's 4 row tiles
                for nt in range(4 * qi, 4 * qi + 4):
                    ysb = y_pool.tile([128, C], F32, tag="y")
                    ntl = (nt - 4 * qi) * 128
                    for cb in range(2):
                        psy = ps_mm.tile([128, QBLK], F32, tag="mm")
                        for pr in range(2):
                            nc.tensor.matmul(
                                psy,
                                lhsT=a2[pr][:, ntl : ntl + 128],
                                rhs=wp2[pr][:, cb * QBLK : (cb + 1) * QBLK],
                                start=(pr == 0),
                                stop=(pr == 1),
                            )
                        nc.vector.tensor_copy(
                            out=ysb[:, cb * QBLK : (cb + 1) * QBLK], in_=psy
                        )
                    nc.sync.dma_start(out=y_d[nt * 128 : (nt + 1) * 128, :], in_=ysb)

    return nc


def _shard_inputs(x, w_qkv, w_proj):
    x = np.ascontiguousarray(np.asarray(x, dtype=np.float32))
    w_qkv = np.asarray(w_qkv, dtype=np.float32)
    w_proj = np.asarray(w_proj, dtype=np.float32)
    xT = [np.ascontiguousarray(x[b].T).reshape(8, 128, N) for b in range(B)]
    in_maps = []
    for c in range(NCORES):
        b, g = divmod(c, 4)
        r0 = 64 * HPC * g  # 256 * g
        wq = w_qkv[r0 : r0 + CSL, :]
        wk = w_qkv[C + r0 : C + r0 + CSL, :]
        wvs = w_qkv[2 * C + r0 : 2 * C + r0 + CSL, :]
        wqkT = np.ascontiguousarray(np.concatenate([wq, wk], axis=0).T)
        wvT = np.ascontiguousarray(wvs.T)
        wpT = np.ascontiguousarray(w_proj[:, r0 : r0 + CSL].T)
        in_maps.append(
            {
                "xT": xT[b],
                "wqkT": wqkT.reshape(8, 128, QKW),
                "wvT": wvT.reshape(8, 128, CSL),
                "wpT": wpT.reshape(2, 128, C),
            }
        )
    return in_maps


def run(x, w_qkv, w_proj, b_proj, trace=False, **spmd_kwargs):
    from concourse.bass_utils import run_bass_kernel_spmd

    in_maps = _shard_inputs(x, w_qkv, w_proj)
    nc = build_nc()
    nc.finalize()
    res = run_bass_kernel_spmd(
        nc, in_maps, core_ids=list(range(NCORES)), trace=trace, **spmd_kwargs
    )
    y = np.zeros((B, N, C), np.float32)
    for c in range(NCORES):
        y[c // 4] += res.results[c]["y"]
    y += np.asarray(b_proj, dtype=np.float32)[None, None, :]
    return y, res


def kernel(x, w_qkv, w_proj, b_proj):
    y, _ = run(x, w_qkv, w_proj, b_proj, trace=False)
    return y


# revision 16
# speedup vs baseline: 1.3169x; 1.0190x over previous
"""Block-causal (block=64) MHA + qkv/out projections on 8 NeuronCores.

Sharding: 8 cores = 2 batches x 4 head-groups (4 heads each).
Per core: qkv projection for its heads, block-causal attention for 4 heads
(processed as 2 head-pairs packed across the 128 partitions), partial output
projection over its 256 channels. Host sums the 4 partials per batch + bias.

On-chip layout is feature-major (transposed): scores are computed transposed
(S^T[k, q] = k . q) so no on-chip transposes are needed anywhere. exp runs on
ScalarE straight out of PSUM.

Matmuls run in float32r (full PE rate). f32r rules honored throughout: every
matmul-operand tile is f32r-typed (producers round on write; memset can't
write f32r, so constants stage through f32 + tensor_copy), and matmul outputs
must start at PSUM partition 0.

The softmax denominator rides in the PV matmul for free: the V tiles are laid
out [v0 | 1 | v1 | v2 | 1 | v3] (384 cols) so each head h has a contiguous
128-col [v_h | ones] / [ones | v_h] stationary window; one full 128x128
matmul per (key tile, head) then yields PV rows on one partition half and the
replicated key-sums on the other. Normalization: reciprocal_approx_fast reads
the sum rows from PSUM, a small SBUF->SBUF DMA shifts the reciprocals onto
the PV lanes, and two tensor_muls write the pair-packed [128, 512] attention
tile, which feeds a 2-chain K=128 output projection.

The diagonal 128-key tiles are one matmul over queries [q0:512) into
persistent per-(j, half) p tiles whose disallowed corner (keys 64:128 x
queries [q0:q0+64)) was zeroed once at startup; the exps write around the
corner, so block-causality costs no per-iteration masking.
"""

import numpy as np

import concourse.bass as bass
import concourse.tile as tile
from concourse import bacc
from concourse import mybir

B, N, C = 2, 2048, 1024
H, HD = 16, 64
HPC = 4  # heads per core
CSL = HPC * HD  # 256 channel slice per core
QKW = 2 * CSL  # 512: q then k output channels
NCORES = 8
QBLK = 512
NQB = N // QBLK  # 4
NT = N // 128  # 16 seq tiles of 128
SCALE = HD**-0.5
F32 = mybir.dt.float32
F32R = mybir.dt.float32r
MMDT = F32R  # matmul-operand dtype: f32r runs the PE at 1 cycle/row


def build_nc():
    nc = bacc.Bacc("TRN2", target_bir_lowering=False, debug=False, num_devices=NCORES)

    xT_d = nc.dram_tensor("xT", [8, 128, N], MMDT, kind="ExternalInput")
    wqk_d = nc.dram_tensor("wqkT", [8, 128, QKW], MMDT, kind="ExternalInput")
    wv_d = nc.dram_tensor("wvT", [8, 128, CSL], MMDT, kind="ExternalInput")
    wp_d = nc.dram_tensor("wpT", [2, 128, C], MMDT, kind="ExternalInput")
    y_d = nc.dram_tensor("y", [N, C], F32, kind="ExternalOutput")

    with tile.TileContext(nc) as tc:
        with (
            tc.tile_pool(name="persist", bufs=1) as persist,
            tc.tile_pool(name="pt", bufs=2) as pt_pool,
            tc.tile_pool(name="rc", bufs=2) as rc_pool,
            tc.tile_pool(name="att", bufs=2) as at_pool,
            tc.tile_pool(name="yout", bufs=2) as y_pool,
            tc.tile_pool(name="psmm", bufs=2, space="PSUM") as ps_mm,
            tc.tile_pool(name="pssc", bufs=2, space="PSUM") as ps_sc,
            tc.tile_pool(name="psacc", bufs=1, space="PSUM") as ps_acc,
        ):
            # ---- load inputs (one tile per DMA so consumers wait on few sems) ----
            xts = [persist.tile([128, N], MMDT, tag=f"xt{i}", name=f"xt{i}") for i in range(8)]
            wqks = [persist.tile([128, QKW], MMDT, tag=f"wqk{i}", name=f"wqk{i}") for i in range(8)]
            wvs = [persist.tile([128, CSL], MMDT, tag=f"wv{i}", name=f"wv{i}") for i in range(8)]
            wp2 = [persist.tile([128, C], MMDT, tag=f"wp{i}", name=f"wp{i}") for i in range(2)]
            for ct in range(8):
                nc.sync.dma_start(out=xts[ct], in_=xT_d[ct])
                nc.sync.dma_start(out=wqks[ct], in_=wqk_d[ct])
                nc.sync.dma_start(out=wvs[ct], in_=wv_d[ct])
            for pr in range(2):
                nc.sync.dma_start(out=wp2[pr], in_=wp_d[pr])

            # memset can't write f32r (ISA check); stage via f32 + rounding copy
            ones_t = persist.tile([128, 128], MMDT, tag="ones")
            ones_f = persist.tile([128, 128], F32, tag="ones_f")
            nc.vector.memset(ones_f, 1.0)
            nc.vector.tensor_copy(out=ones_t, in_=ones_f)
            zero_r = persist.tile([128, 64], MMDT, tag="zero_r")
            zero_f = persist.tile([128, 64], F32, tag="zero_f")
            nc.vector.memset(zero_f, 0.0)
            nc.vector.tensor_copy(out=zero_r, in_=zero_f)

            # persistent diagonal p tiles, one per (j, head-half). Their
            # disallowed corner (keys 64:128 x queries [128j, 128j+64)) is
            # zeroed ONCE here; the diag exps write around it, so PV/sum can
            # read the full [*, q0:512) range with the corner always zero.
            pd = {}
            for j in range(4):
                for hh in range(2):
                    t = persist.tile(
                        [128, QBLK], MMDT, tag=f"pd{j}{hh}", name=f"pd{j}{hh}"
                    )
                    pd[j, hh] = t
                    nc.gpsimd.tensor_copy(
                        out=t[64:128, 128 * j : 128 * j + 64], in_=zero_r[64:128, :]
                    )

            # ---- phase 1: q/k projection, transposed outputs ----
            # qkT tiles: 0 = q heads(0,1), 1 = q heads(2,3), 2 = k(0,1), 3 = k(2,3)
            # within a tile: partitions 0:64 = first head dims, 64:128 = second.
            qkT = [persist.tile([128, N], MMDT, tag=f"qk{t}", name=f"qk{t}") for t in range(4)]
            for dt_ in range(4):
                for nb in range(NQB):
                    ps = ps_mm.tile([128, QBLK], F32, tag="mm")
                    for ct in range(8):
                        nc.tensor.matmul(
                            ps,
                            lhsT=wqks[ct][:, dt_ * 128 : (dt_ + 1) * 128],
                            rhs=xts[ct][:, nb * QBLK : (nb + 1) * QBLK],
                            start=(ct == 0),
                            stop=(ct == 7),
                        )
                    nc.vector.tensor_copy(
                        out=qkT[dt_][:, nb * QBLK : (nb + 1) * QBLK], in_=ps
                    )

            # ---- phase 2: v projection into [v0 | 1 | v1 | v2 | 1 | v3] ----
            # head h's PV stationary is the 128-col window starting at 64*h
            # offset... concretely: head 0 -> cols 0:128 ([v0|1]), head 1 ->
            # cols 64:192 ([1|v1]), head 2 -> 192:320 ([v2|1]), head 3 ->
            # 256:384 ([1|v3]).
            v65 = [persist.tile([128, 384], MMDT, tag=f"v{t}", name=f"v{t}") for t in range(NT)]
            for nt in range(NT):
                nc.gpsimd.tensor_copy(out=v65[nt][:, 64:128], in_=ones_t[:, 0:64])
                nc.gpsimd.tensor_copy(out=v65[nt][:, 256:320], in_=ones_t[:, 0:64])
                ps = ps_mm.tile([128, CSL], F32, tag="mm")
                for ct in range(8):
                    nc.tensor.matmul(
                        ps,
                        lhsT=xts[ct][:, nt * 128 : (nt + 1) * 128],
                        rhs=wvs[ct],
                        start=(ct == 0),
                        stop=(ct == 7),
                    )
                nc.vector.tensor_copy(out=v65[nt][:, 0:64], in_=ps[:, 0:64])
                nc.vector.tensor_copy(out=v65[nt][:, 128:256], in_=ps[:, 64:192])
                nc.vector.tensor_copy(out=v65[nt][:, 320:384], in_=ps[:, 192:256])

            # stationary windows: (even head A, odd head B) per pair
            def vwin(kt, pair, hh):
                base = pair * 192 + hh * 64
                return v65[kt][:, base : base + 128]

            # ---- phase 3+4: attention (per 512-query block), then out-proj ----
            def emit_outproj(qi, a2):
                # output projection for query block qi's 4 row tiles
                for nt in range(4 * qi, 4 * qi + 4):
                    ysb = y_pool.tile([128, C], F32, tag="y", name=f"ysb{nt}")
                    ntl = (nt - 4 * qi) * 128
                    for cb in range(2):
                        psy = ps_mm.tile([128, QBLK], F32, tag="mm", name="psy")
                        for pr in range(2):
                            nc.tensor.matmul(
                                psy,
                                lhsT=a2[pr][:, ntl : ntl + 128],
                                rhs=wp2[pr][:, cb * QBLK : (cb + 1) * QBLK],
                                start=(pr == 0),
                                stop=(pr == 1),
                            )
                        nc.vector.tensor_copy(
                            out=ysb[:, cb * QBLK : (cb + 1) * QBLK], in_=psy
                        )
                    nc.sync.dma_start(out=y_d[nt * 128 : (nt + 1) * 128, :], in_=ysb)

            pending = None  # (qi, a2) whose out-proj is deferred one block
            for qi in range(NQB):
                a2 = [
                    at_pool.tile([128, QBLK], MMDT, tag=f"a{p}", name=f"a{p}_{qi}")
                    for p in range(2)
                ]
                for pair in range(2):
                    qt = qkT[pair]
                    kt_t = qkT[2 + pair]
                    qs = slice(qi * QBLK, (qi + 1) * QBLK)

                    # one PSUM bank per head: PV rows on one partition half,
                    # replicated key-sums on the other (from the ones block
                    # of the stationary). A: PV 0:64 / sums 64:128; B: sums
                    # 0:64 / PV 64:128.
                    at_bA = ps_acc.tile([128, QBLK], F32, tag="atA", name="at_bA")
                    at_bB = ps_acc.tile([128, QBLK], F32, tag="atB", name="at_bB")

                    n_reg = 4 * qi
                    n_per_range = n_reg + 4
                    at_A, at_B = [0], [0]

                    def fl(cnt, total=n_per_range):
                        i = cnt[0]
                        cnt[0] += 1
                        return dict(start=(i == 0), stop=(i == total - 1))

                    # work items: rect key tiles then diagonal tiles
                    items = [("r", kt) for kt in range(n_reg)]
                    items += [("d", j) for j in range(4)]

                    def emit_scores(item):
                        """score matmuls + exps for one key tile; returns the
                        p tiles + query range for the later PV matmuls."""
                        kind, idx = item
                        psA = ps_sc.tile([128, QBLK], F32, tag="sA", name="psA")
                        psB = ps_sc.tile([128, QBLK], F32, tag="sB", name="psB")
                        if kind == "r":
                            kt, q0 = idx, 0
                            ks = slice(kt * 128, (kt + 1) * 128)
                            nc.tensor.matmul(
                                psA, lhsT=kt_t[0:64, ks], rhs=qt[0:64, qs],
                                start=True, stop=True,
                            )
                            nc.tensor.matmul(
                                psB, lhsT=kt_t[64:128, ks], rhs=qt[64:128, qs],
                                start=True, stop=True,
                            )
                            pA = pt_pool.tile([128, QBLK], MMDT, tag="pA", name="pA")
                            pB = pt_pool.tile([128, QBLK], MMDT, tag="pB", name="pB")
                            nc.scalar.activation(
                                out=pA, in_=psA,
                                func=mybir.ActivationFunctionType.Exp, scale=SCALE,
                            )
                            nc.scalar.activation(
                                out=pB, in_=psB,
                                func=mybir.ActivationFunctionType.Exp, scale=SCALE,
                            )
                        else:
                            j = idx
                            kt = 4 * qi + j
                            q0 = 128 * j
                            q1 = q0 + 64
                            ksl = slice(kt * 128, (kt + 1) * 128)
                            qsl = slice(qi * QBLK + q0, (qi + 1) * QBLK)
                            pA = pd[j, 0]
                            pB = pd[j, 1]
                            for ph, ps_s, p_s in ((0, psA, pA), (64, psB, pB)):
                                hd_sl = slice(ph, ph + 64)
                                nc.tensor.matmul(
                                    ps_s[:, q0:QBLK], lhsT=kt_t[hd_sl, ksl],
                                    rhs=qt[hd_sl, qsl], start=True, stop=True,
                                )
                                # two exps that skirt the pre-zeroed corner
                                nc.scalar.activation(
                                    out=p_s[0:64, q0:QBLK], in_=ps_s[0:64, q0:QBLK],
                                    func=mybir.ActivationFunctionType.Exp,
                                    scale=SCALE,
                                )
                                nc.scalar.activation(
                                    out=p_s[64:128, q1:QBLK],
                                    in_=ps_s[64:128, q1:QBLK],
                                    func=mybir.ActivationFunctionType.Exp,
                                    scale=SCALE,
                                )
                        return kt, q0, pA, pB

                    def emit_pv(staged):
                        kt, q0, pA, pB = staged
                        nc.tensor.matmul(
                            at_bA[:, q0:QBLK], lhsT=vwin(kt, pair, 0),
                            rhs=pA[:, q0:QBLK], **fl(at_A),
                        )
                        nc.tensor.matmul(
                            at_bB[:, q0:QBLK], lhsT=vwin(kt, pair, 1),
                            rhs=pB[:, q0:QBLK], **fl(at_B),
                        )

                    # software-pipelined: scores run one key tile ahead of
                    # PV so the exp latency hides under the next scores; the
                    # deferred out-proj of the previous query block slots in
                    # behind the first few tiles.
                    staged = emit_scores(items[0]) if items else None
                    for i in range(len(items)):
                        nxt = emit_scores(items[i + 1]) if i + 1 < len(items) else None
                        if pair == 0 and i == min(2, len(items) - 1) and pending:
                            emit_outproj(*pending)
                            pending = None
                        emit_pv(staged)
                        staged = nxt

                    # normalize: copy/reciprocal out of PSUM, DMA shifts the
                    # values onto the PV lanes, fast reciprocal for A's half
                    # (custom-DVE ops misread PSUM base 64, so A's sums are
                    # copied out and reciprocated after the shift)
                    rec = rc_pool.tile([128, QBLK], F32, tag="rec")
                    rsh = rc_pool.tile([128, QBLK], F32, tag="rsh")
                    rcf = rc_pool.tile([128, QBLK], F32, tag="rcf")
                    nc.vector.tensor_copy(
                        out=rec[64:128, :], in_=at_bA[64:128, :]
                    )
                    nc.vector.reciprocal_approx_fast(
                        out=rec[0:64, :], in_=at_bB[0:64, :]
                    )
                    nc.sync.dma_start(out=rsh[0:64, :], in_=rec[64:128, :])
                    nc.sync.dma_start(out=rsh[64:128, :], in_=rec[0:64, :])
                    nc.vector.reciprocal_approx_fast(
                        out=rcf[0:64, :], in_=rsh[0:64, :]
                    )
                    nc.vector.tensor_mul(
                        out=a2[pair][0:64, :], in0=at_bA[0:64, :], in1=rcf[0:64, :]
                    )
                    nc.vector.tensor_mul(
                        out=a2[pair][64:128, :], in0=at_bB[64:128, :],
                        in1=rsh[64:128, :],
                    )

                if pending:  # qi=0 has few tiles; flush if not yet emitted
                    emit_outproj(*pending)
                pending = (qi, a2)
            emit_outproj(*pending)

    return nc


def _shard_inputs(x, w_qkv, w_proj):
    x = np.ascontiguousarray(np.asarray(x, dtype=np.float32))
    w_qkv = np.asarray(w_qkv, dtype=np.float32)
    w_proj = np.asarray(w_proj, dtype=np.float32)
    xT = [np.ascontiguousarray(x[b].T).reshape(8, 128, N) for b in range(B)]
    in_maps = []
    for c in range(NCORES):
        b, g = divmod(c, 4)
        r0 = 64 * HPC * g  # 256 * g
        wq = w_qkv[r0 : r0 + CSL, :]
        wk = w_qkv[C + r0 : C + r0 + CSL, :]
        wvs = w_qkv[2 * C + r0 : 2 * C + r0 + CSL, :]
        wqkT = np.ascontiguousarray(np.concatenate([wq, wk], axis=0).T)
        wvT = np.ascontiguousarray(wvs.T)
        wpT = np.ascontiguousarray(w_proj[:, r0 : r0 + CSL].T)
        in_maps.append(
            {
                "xT": xT[b],
                "wqkT": wqkT.reshape(8, 128, QKW),
                "wvT": wvT.reshape(8, 128, CSL),
                "wpT": wpT.reshape(2, 128, C),
            }
        )
    return in_maps


def run(x, w_qkv, w_proj, b_proj, trace=False, **spmd_kwargs):
    from concourse.bass_utils import run_bass_kernel_spmd

    in_maps = _shard_inputs(x, w_qkv, w_proj)
    nc = build_nc()
    nc.finalize()
    res = run_bass_kernel_spmd(
        nc, in_maps, core_ids=list(range(NCORES)), trace=trace, **spmd_kwargs
    )
    y = np.zeros((B, N, C), np.float32)
    for c in range(NCORES):
        y[c // 4] += res.results[c]["y"]
    y += np.asarray(b_proj, dtype=np.float32)[None, None, :]
    return y, res


def kernel(x, w_qkv, w_proj, b_proj):
    y, _ = run(x, w_qkv, w_proj, b_proj, trace=False)
    return y


# revision 17
# speedup vs baseline: 1.6411x; 1.2462x over previous
"""Block-causal (block=64) MHA + qkv/out projections on 8 NeuronCores.

Sharding: 8 cores = 2 batches x 4 head-groups (4 heads each).
Per core: qkv projection for its heads, block-causal attention for 4 heads
(processed as 2 head-pairs packed across the 128 partitions), partial output
projection over its 256 channels. Host sums the 4 partials per batch + bias.

On-chip layout is feature-major (transposed): scores are computed transposed
(S^T[k, q] = k . q) so no on-chip transposes are needed anywhere. exp runs on
ScalarE straight out of PSUM.

Matmuls run in float32r (full PE rate). f32r rules honored throughout: every
matmul-operand tile is f32r-typed (producers round on write; memset can't
write f32r, so constants stage through f32 + tensor_copy), and matmul outputs
must start at PSUM partition 0.

The softmax denominator rides in the PV matmul for free: the V tiles are laid
out [v0 | 1 | v1 | v2 | 1 | v3] (384 cols) so each head h has a contiguous
128-col [v_h | ones] / [ones | v_h] stationary window; one full 128x128
matmul per (key tile, head) then yields PV rows on one partition half and the
replicated key-sums on the other. Normalization: reciprocal_approx_fast reads
the sum rows from PSUM, a small SBUF->SBUF DMA shifts the reciprocals onto
the PV lanes, and two tensor_muls write the pair-packed [128, 512] attention
tile, which feeds a 2-chain K=128 output projection.

The diagonal 128-key tiles are one matmul over queries [q0:512) into
persistent per-(j, half) p tiles whose disallowed corner (keys 64:128 x
queries [q0:q0+64)) was zeroed once at startup; the exps write around the
corner, so block-causality costs no per-iteration masking.
"""

import numpy as np

import concourse.bass as bass
import concourse.tile as tile
from concourse import bacc
from concourse import mybir

B, N, C = 2, 2048, 1024
H, HD = 16, 64
HPC = 4  # heads per core
CSL = HPC * HD  # 256 channel slice per core
QKW = 2 * CSL  # 512: q then k output channels
NCORES = 8
QBLK = 512
NQB = N // QBLK  # 4
NT = N // 128  # 16 seq tiles of 128
SCALE = HD**-0.5
F32 = mybir.dt.float32
F32R = mybir.dt.float32r
BF16 = mybir.dt.bfloat16
MMDT = BF16  # matmul-operand dtype: bf16 runs the PE at full rate


def build_nc():
    nc = bacc.Bacc("TRN2", target_bir_lowering=False, debug=False, num_devices=NCORES)

    xT_d = nc.dram_tensor("xT", [8, 128, N], MMDT, kind="ExternalInput")
    wqk_d = nc.dram_tensor("wqkT", [8, 128, QKW], MMDT, kind="ExternalInput")
    wv_d = nc.dram_tensor("wvT", [8, 128, CSL], MMDT, kind="ExternalInput")
    wp_d = nc.dram_tensor("wpT", [2, 128, C], MMDT, kind="ExternalInput")
    y_d = nc.dram_tensor("y", [N, C], F32, kind="ExternalOutput")

    with tile.TileContext(nc) as tc:
        with (
            tc.tile_pool(name="persist", bufs=1) as persist,
            tc.tile_pool(name="pt", bufs=2) as pt_pool,
            tc.tile_pool(name="rc", bufs=2) as rc_pool,
            tc.tile_pool(name="att", bufs=2) as at_pool,
            tc.tile_pool(name="yout", bufs=2) as y_pool,
            tc.tile_pool(name="psmm", bufs=2, space="PSUM") as ps_mm,
            tc.tile_pool(name="pssc", bufs=2, space="PSUM") as ps_sc,
            tc.tile_pool(name="psacc", bufs=1, space="PSUM") as ps_acc,
        ):
            # ---- load inputs (one tile per DMA so consumers wait on few sems) ----
            xts = [persist.tile([128, N], MMDT, tag=f"xt{i}", name=f"xt{i}") for i in range(8)]
            wqks = [persist.tile([128, QKW], MMDT, tag=f"wqk{i}", name=f"wqk{i}") for i in range(8)]
            wvs = [persist.tile([128, CSL], MMDT, tag=f"wv{i}", name=f"wv{i}") for i in range(8)]
            wp2 = [persist.tile([128, C], MMDT, tag=f"wp{i}", name=f"wp{i}") for i in range(2)]
            for ct in range(8):
                nc.sync.dma_start(out=xts[ct], in_=xT_d[ct])
                nc.sync.dma_start(out=wqks[ct], in_=wqk_d[ct])
                nc.sync.dma_start(out=wvs[ct], in_=wv_d[ct])
            for pr in range(2):
                nc.sync.dma_start(out=wp2[pr], in_=wp_d[pr])

            # memset can't write f32r (ISA check); stage via f32 + rounding copy
            ones_t = persist.tile([128, 128], MMDT, tag="ones")
            ones_f = persist.tile([128, 128], F32, tag="ones_f")
            nc.vector.memset(ones_f, 1.0)
            nc.vector.tensor_copy(out=ones_t, in_=ones_f)
            zero_r = persist.tile([128, 64], MMDT, tag="zero_r")
            zero_f = persist.tile([128, 64], F32, tag="zero_f")
            nc.vector.memset(zero_f, 0.0)
            nc.vector.tensor_copy(out=zero_r, in_=zero_f)

            # persistent diagonal p tiles, one per (j, head-half). Their
            # disallowed corner (keys 64:128 x queries [128j, 128j+64)) is
            # zeroed ONCE here; the diag exps write around it, so PV/sum can
            # read the full [*, q0:512) range with the corner always zero.
            pd = {}
            for j in range(4):
                for hh in range(2):
                    t = persist.tile(
                        [128, QBLK], MMDT, tag=f"pd{j}{hh}", name=f"pd{j}{hh}"
                    )
                    pd[j, hh] = t
                    nc.gpsimd.tensor_copy(
                        out=t[64:128, 128 * j : 128 * j + 64], in_=zero_r[64:128, :]
                    )

            # ---- phase 1: q/k projection, transposed outputs ----
            # qkT tiles: 0 = q heads(0,1), 1 = q heads(2,3), 2 = k(0,1), 3 = k(2,3)
            # within a tile: partitions 0:64 = first head dims, 64:128 = second.
            qkT = [persist.tile([128, N], MMDT, tag=f"qk{t}", name=f"qk{t}") for t in range(4)]
            for dt_ in range(4):
                for nb in range(NQB):
                    ps = ps_mm.tile([128, QBLK], F32, tag="mm")
                    for ct in range(8):
                        nc.tensor.matmul(
                            ps,
                            lhsT=wqks[ct][:, dt_ * 128 : (dt_ + 1) * 128],
                            rhs=xts[ct][:, nb * QBLK : (nb + 1) * QBLK],
                            start=(ct == 0),
                            stop=(ct == 7),
                        )
                    nc.vector.tensor_copy(
                        out=qkT[dt_][:, nb * QBLK : (nb + 1) * QBLK], in_=ps
                    )

            # ---- phase 2: v projection into [v0 | 1 | v1 | v2 | 1 | v3] ----
            # head h's PV stationary is the 128-col window starting at 64*h
            # offset... concretely: head 0 -> cols 0:128 ([v0|1]), head 1 ->
            # cols 64:192 ([1|v1]), head 2 -> 192:320 ([v2|1]), head 3 ->
            # 256:384 ([1|v3]).
            v65 = [persist.tile([128, 384], MMDT, tag=f"v{t}", name=f"v{t}") for t in range(NT)]
            for nt in range(NT):
                nc.gpsimd.tensor_copy(out=v65[nt][:, 64:128], in_=ones_t[:, 0:64])
                nc.gpsimd.tensor_copy(out=v65[nt][:, 256:320], in_=ones_t[:, 0:64])
                ps = ps_mm.tile([128, CSL], F32, tag="mm")
                for ct in range(8):
                    nc.tensor.matmul(
                        ps,
                        lhsT=xts[ct][:, nt * 128 : (nt + 1) * 128],
                        rhs=wvs[ct],
                        start=(ct == 0),
                        stop=(ct == 7),
                    )
                nc.vector.tensor_copy(out=v65[nt][:, 0:64], in_=ps[:, 0:64])
                nc.vector.tensor_copy(out=v65[nt][:, 128:256], in_=ps[:, 64:192])
                nc.vector.tensor_copy(out=v65[nt][:, 320:384], in_=ps[:, 192:256])

            # stationary windows: (even head A, odd head B) per pair
            def vwin(kt, pair, hh):
                base = pair * 192 + hh * 64
                return v65[kt][:, base : base + 128]

            # ---- phase 3+4: attention (per 512-query block), then out-proj ----
            def emit_outproj(qi, a2):
                # output projection for query block qi's 4 row tiles
                for nt in range(4 * qi, 4 * qi + 4):
                    ysb = y_pool.tile([128, C], F32, tag="y", name=f"ysb{nt}")
                    ntl = (nt - 4 * qi) * 128
                    for cb in range(2):
                        psy = ps_mm.tile([128, QBLK], F32, tag="mm", name="psy")
                        for pr in range(2):
                            nc.tensor.matmul(
                                psy,
                                lhsT=a2[pr][:, ntl : ntl + 128],
                                rhs=wp2[pr][:, cb * QBLK : (cb + 1) * QBLK],
                                start=(pr == 0),
                                stop=(pr == 1),
                            )
                        nc.vector.tensor_copy(
                            out=ysb[:, cb * QBLK : (cb + 1) * QBLK], in_=psy
                        )
                    nc.sync.dma_start(out=y_d[nt * 128 : (nt + 1) * 128, :], in_=ysb)

            pending = None  # (qi, a2) whose out-proj is deferred one block
            for qi in range(NQB):
                a2 = [
                    at_pool.tile([128, QBLK], MMDT, tag=f"a{p}", name=f"a{p}_{qi}")
                    for p in range(2)
                ]
                for pair in range(2):
                    qt = qkT[pair]
                    kt_t = qkT[2 + pair]
                    qs = slice(qi * QBLK, (qi + 1) * QBLK)

                    # one PSUM bank per head: PV rows on one partition half,
                    # replicated key-sums on the other (from the ones block
                    # of the stationary). A: PV 0:64 / sums 64:128; B: sums
                    # 0:64 / PV 64:128.
                    at_bA = ps_acc.tile([128, QBLK], F32, tag="atA", name="at_bA")
                    at_bB = ps_acc.tile([128, QBLK], F32, tag="atB", name="at_bB")

                    n_reg = 4 * qi
                    n_per_range = n_reg + 4
                    at_A, at_B = [0], [0]

                    def fl(cnt, total=n_per_range):
                        i = cnt[0]
                        cnt[0] += 1
                        return dict(start=(i == 0), stop=(i == total - 1))

                    # work items: rect key tiles then diagonal tiles
                    items = [("r", kt) for kt in range(n_reg)]
                    items += [("d", j) for j in range(4)]

                    def emit_scores(item):
                        """score matmuls + exps for one key tile; returns the
                        p tiles + query range for the later PV matmuls."""
                        kind, idx = item
                        psA = ps_sc.tile([128, QBLK], F32, tag="sA", name="psA")
                        psB = ps_sc.tile([128, QBLK], F32, tag="sB", name="psB")
                        if kind == "r":
                            kt, q0 = idx, 0
                            ks = slice(kt * 128, (kt + 1) * 128)
                            nc.tensor.matmul(
                                psA, lhsT=kt_t[0:64, ks], rhs=qt[0:64, qs],
                                start=True, stop=True,
                            )
                            nc.tensor.matmul(
                                psB, lhsT=kt_t[64:128, ks], rhs=qt[64:128, qs],
                                start=True, stop=True,
                            )
                            pA = pt_pool.tile([128, QBLK], MMDT, tag="pA", name="pA")
                            pB = pt_pool.tile([128, QBLK], MMDT, tag="pB", name="pB")
                            nc.scalar.activation(
                                out=pA, in_=psA,
                                func=mybir.ActivationFunctionType.Exp, scale=SCALE,
                            )
                            nc.scalar.activation(
                                out=pB, in_=psB,
                                func=mybir.ActivationFunctionType.Exp, scale=SCALE,
                            )
                        else:
                            j = idx
                            kt = 4 * qi + j
                            q0 = 128 * j
                            q1 = q0 + 64
                            ksl = slice(kt * 128, (kt + 1) * 128)
                            qsl = slice(qi * QBLK + q0, (qi + 1) * QBLK)
                            pA = pd[j, 0]
                            pB = pd[j, 1]
                            for ph, ps_s, p_s in ((0, psA, pA), (64, psB, pB)):
                                hd_sl = slice(ph, ph + 64)
                                nc.tensor.matmul(
                                    ps_s[:, q0:QBLK], lhsT=kt_t[hd_sl, ksl],
                                    rhs=qt[hd_sl, qsl], start=True, stop=True,
                                )
                                # two exps that skirt the pre-zeroed corner
                                nc.scalar.activation(
                                    out=p_s[0:64, q0:QBLK], in_=ps_s[0:64, q0:QBLK],
                                    func=mybir.ActivationFunctionType.Exp,
                                    scale=SCALE,
                                )
                                nc.scalar.activation(
                                    out=p_s[64:128, q1:QBLK],
                                    in_=ps_s[64:128, q1:QBLK],
                                    func=mybir.ActivationFunctionType.Exp,
                                    scale=SCALE,
                                )
                        return kt, q0, pA, pB

                    def emit_pv(staged):
                        kt, q0, pA, pB = staged
                        nc.tensor.matmul(
                            at_bA[:, q0:QBLK], lhsT=vwin(kt, pair, 0),
                            rhs=pA[:, q0:QBLK], **fl(at_A),
                        )
                        nc.tensor.matmul(
                            at_bB[:, q0:QBLK], lhsT=vwin(kt, pair, 1),
                            rhs=pB[:, q0:QBLK], **fl(at_B),
                        )

                    # software-pipelined: scores run one key tile ahead of
                    # PV so the exp latency hides under the next scores; the
                    # deferred out-proj of the previous query block slots in
                    # behind the first few tiles.
                    staged = emit_scores(items[0]) if items else None
                    for i in range(len(items)):
                        nxt = emit_scores(items[i + 1]) if i + 1 < len(items) else None
                        if pair == 0 and i == min(2, len(items) - 1) and pending:
                            emit_outproj(*pending)
                            pending = None
                        emit_pv(staged)
                        staged = nxt

                    # normalize: copy/reciprocal out of PSUM, DMA shifts the
                    # values onto the PV lanes, fast reciprocal for A's half
                    # (custom-DVE ops misread PSUM base 64, so A's sums are
                    # copied out and reciprocated after the shift)
                    rec = rc_pool.tile([128, QBLK], F32, tag="rec")
                    rsh = rc_pool.tile([128, QBLK], F32, tag="rsh")
                    rcf = rc_pool.tile([128, QBLK], F32, tag="rcf")
                    nc.vector.tensor_copy(
                        out=rec[64:128, :], in_=at_bA[64:128, :]
                    )
                    nc.vector.reciprocal_approx_fast(
                        out=rec[0:64, :], in_=at_bB[0:64, :]
                    )
                    nc.sync.dma_start(out=rsh[0:64, :], in_=rec[64:128, :])
                    nc.sync.dma_start(out=rsh[64:128, :], in_=rec[0:64, :])
                    nc.vector.reciprocal_approx_fast(
                        out=rcf[0:64, :], in_=rsh[0:64, :]
                    )
                    nc.vector.tensor_mul(
                        out=a2[pair][0:64, :], in0=at_bA[0:64, :], in1=rcf[0:64, :]
                    )
                    nc.vector.tensor_mul(
                        out=a2[pair][64:128, :], in0=at_bB[64:128, :],
                        in1=rsh[64:128, :],
                    )

                if pending:  # qi=0 has few tiles; flush if not yet emitted
                    emit_outproj(*pending)
                pending = (qi, a2)
            emit_outproj(*pending)

    return nc


def _shard_inputs(x, w_qkv, w_proj):
    import ml_dtypes

    bf16 = ml_dtypes.bfloat16
    x = np.ascontiguousarray(np.asarray(x, dtype=np.float32).astype(bf16))
    w_qkv = np.asarray(w_qkv, dtype=np.float32).astype(bf16)
    w_proj = np.asarray(w_proj, dtype=np.float32).astype(bf16)
    xT = [np.ascontiguousarray(x[b].T).reshape(8, 128, N) for b in range(B)]
    in_maps = []
    for c in range(NCORES):
        b, g = divmod(c, 4)
        r0 = 64 * HPC * g  # 256 * g
        wq = w_qkv[r0 : r0 + CSL, :]
        wk = w_qkv[C + r0 : C + r0 + CSL, :]
        wvs = w_qkv[2 * C + r0 : 2 * C + r0 + CSL, :]
        wqkT = np.ascontiguousarray(np.concatenate([wq, wk], axis=0).T)
        wvT = np.ascontiguousarray(wvs.T)
        wpT = np.ascontiguousarray(w_proj[:, r0 : r0 + CSL].T)
        in_maps.append(
            {
                "xT": xT[b],
                "wqkT": wqkT.reshape(8, 128, QKW),
                "wvT": wvT.reshape(8, 128, CSL),
                "wpT": wpT.reshape(2, 128, C),
            }
        )
    return in_maps


def run(x, w_qkv, w_proj, b_proj, trace=False, **spmd_kwargs):
    from concourse.bass_utils import run_bass_kernel_spmd

    in_maps = _shard_inputs(x, w_qkv, w_proj)
    nc = build_nc()
    nc.finalize()
    res = run_bass_kernel_spmd(
        nc, in_maps, core_ids=list(range(NCORES)), trace=trace, **spmd_kwargs
    )
    y = np.zeros((B, N, C), np.float32)
    for c in range(NCORES):
        y[c // 4] += res.results[c]["y"]
    y += np.asarray(b_proj, dtype=np.float32)[None, None, :]
    return y, res


def kernel(x, w_qkv, w_proj, b_proj):
    y, _ = run(x, w_qkv, w_proj, b_proj, trace=False)
    return y


# revision 20
# speedup vs baseline: 1.7851x; 1.0878x over previous
"""Block-causal (block=64) MHA + qkv/out projections on 8 NeuronCores.

Sharding: 8 cores = 2 batches x 4 head-groups (4 heads each).
Per core: qkv projection for its heads, block-causal attention for 4 heads
(processed as 2 head-pairs packed across the 128 partitions), partial output
projection over its 256 channels. Host sums the 4 partials per batch + bias.

On-chip layout is feature-major (transposed): scores are computed transposed
(S^T[k, q] = k . q) so no on-chip transposes are needed anywhere. exp runs on
ScalarE straight out of PSUM.

Matmuls run in float32r (full PE rate). f32r rules honored throughout: every
matmul-operand tile is f32r-typed (producers round on write; memset can't
write f32r, so constants stage through f32 + tensor_copy), and matmul outputs
must start at PSUM partition 0.

The softmax denominator rides in the PV matmul for free: the V tiles are laid
out [v0 | 1 | v1 | v2 | 1 | v3] (384 cols) so each head h has a contiguous
128-col [v_h | ones] / [ones | v_h] stationary window; one full 128x128
matmul per (key tile, head) then yields PV rows on one partition half and the
replicated key-sums on the other. Normalization: reciprocal_approx_fast reads
the sum rows from PSUM, a small SBUF->SBUF DMA shifts the reciprocals onto
the PV lanes, and two tensor_muls write the pair-packed [128, 512] attention
tile, which feeds a 2-chain K=128 output projection.

The diagonal 128-key tiles are one matmul over queries [q0:512) into
persistent per-(j, half) p tiles whose disallowed corner (keys 64:128 x
queries [q0:q0+64)) was zeroed once at startup; the exps write around the
corner, so block-causality costs no per-iteration masking.
"""

import numpy as np

import concourse.bass as bass
import concourse.tile as tile
from concourse import bacc
from concourse import mybir

B, N, C = 2, 2048, 1024
H, HD = 16, 64
HPC = 4  # heads per core
CSL = HPC * HD  # 256 channel slice per core
QKW = 2 * CSL  # 512: q then k output channels
NCORES = 8
QBLK = 512
NQB = N // QBLK  # 4
NT = N // 128  # 16 seq tiles of 128
SCALE = HD**-0.5
F32 = mybir.dt.float32
F32R = mybir.dt.float32r
BF16 = mybir.dt.bfloat16
MMDT = BF16  # matmul-operand dtype: bf16 runs the PE at full rate


def build_nc():
    nc = bacc.Bacc("TRN2", target_bir_lowering=False, debug=False, num_devices=NCORES)

    xT_d = nc.dram_tensor("xT", [8, 128, N], MMDT, kind="ExternalInput")
    wqk_d = nc.dram_tensor("wqkT", [8, 128, QKW], MMDT, kind="ExternalInput")
    wv_d = nc.dram_tensor("wvT", [8, 128, CSL], MMDT, kind="ExternalInput")
    wp_d = nc.dram_tensor("wpT", [2, 128, C], MMDT, kind="ExternalInput")
    y_d = nc.dram_tensor("y", [N, C], F32, kind="ExternalOutput")

    with tile.TileContext(nc) as tc:
        with (
            tc.tile_pool(name="persist", bufs=1) as persist,
            tc.tile_pool(name="pt", bufs=3) as pt_pool,
            tc.tile_pool(name="rc", bufs=2) as rc_pool,
            tc.tile_pool(name="att", bufs=2) as at_pool,
            tc.tile_pool(name="yout", bufs=2) as y_pool,
            tc.tile_pool(name="psmm", bufs=2, space="PSUM") as ps_mm,
            tc.tile_pool(name="pssc", bufs=2, space="PSUM") as ps_sc,
            tc.tile_pool(name="psacc", bufs=1, space="PSUM") as ps_acc,
        ):
            # ---- load inputs (one tile per DMA so consumers wait on few sems) ----
            xts = [persist.tile([128, N], MMDT, tag=f"xt{i}", name=f"xt{i}") for i in range(8)]
            wqks = [persist.tile([128, QKW], MMDT, tag=f"wqk{i}", name=f"wqk{i}") for i in range(8)]
            wvs = [persist.tile([128, CSL], MMDT, tag=f"wv{i}", name=f"wv{i}") for i in range(8)]
            wp2 = [persist.tile([128, C], MMDT, tag=f"wp{i}", name=f"wp{i}") for i in range(2)]
            # issue in phase-1 consumption order: (x, wqk) pairs first so the
            # first projection chain starts ~2us in; v/proj weights later
            for ct in range(8):
                nc.sync.dma_start(out=xts[ct], in_=xT_d[ct])
                nc.sync.dma_start(out=wqks[ct], in_=wqk_d[ct])
            for ct in range(8):
                nc.sync.dma_start(out=wvs[ct], in_=wv_d[ct])
            for pr in range(2):
                nc.sync.dma_start(out=wp2[pr], in_=wp_d[pr])

            # memset can't write f32r (ISA check); stage via f32 + rounding copy
            ones_t = persist.tile([128, 128], MMDT, tag="ones")
            ones_f = persist.tile([128, 128], F32, tag="ones_f")
            nc.vector.memset(ones_f, 1.0)
            nc.vector.tensor_copy(out=ones_t, in_=ones_f)
            zero_r = persist.tile([128, 64], MMDT, tag="zero_r")
            zero_f = persist.tile([128, 64], F32, tag="zero_f")
            nc.vector.memset(zero_f, 0.0)
            nc.vector.tensor_copy(out=zero_r, in_=zero_f)

            # persistent diagonal p tiles, one per (j, head-half). Their
            # disallowed corner (keys 64:128 x queries [128j, 128j+64)) is
            # zeroed ONCE here; the diag exps write around it, so PV/sum can
            # read the full [*, q0:512) range with the corner always zero.
            pd = {}
            for j in range(4):
                for hh in range(2):
                    t = persist.tile(
                        [128, QBLK], MMDT, tag=f"pd{j}{hh}", name=f"pd{j}{hh}"
                    )
                    pd[j, hh] = t
                    nc.gpsimd.tensor_copy(
                        out=t[64:128, 128 * j : 128 * j + 64], in_=zero_r[64:128, :]
                    )

            # ---- phase 1: q/k projection, transposed outputs ----
            # qkT tiles: 0 = q heads(0,1), 1 = q heads(2,3), 2 = k(0,1), 3 = k(2,3)
            # within a tile: partitions 0:64 = first head dims, 64:128 = second.
            qkT = [persist.tile([128, N], MMDT, tag=f"qk{t}", name=f"qk{t}") for t in range(4)]
            for dt_ in range(4):
                for nb in range(NQB):
                    ps = ps_mm.tile([128, QBLK], F32, tag="mm")
                    for ct in range(8):
                        nc.tensor.matmul(
                            ps,
                            lhsT=wqks[ct][:, dt_ * 128 : (dt_ + 1) * 128],
                            rhs=xts[ct][:, nb * QBLK : (nb + 1) * QBLK],
                            start=(ct == 0),
                            stop=(ct == 7),
                        )
                    nc.vector.tensor_copy(
                        out=qkT[dt_][:, nb * QBLK : (nb + 1) * QBLK], in_=ps
                    )

            # ---- phase 2: v projection into [v0 | 1 | v1 | v2 | 1 | v3] ----
            # head h's PV stationary is the 128-col window starting at 64*h
            # offset... concretely: head 0 -> cols 0:128 ([v0|1]), head 1 ->
            # cols 64:192 ([1|v1]), head 2 -> 192:320 ([v2|1]), head 3 ->
            # 256:384 ([1|v3]).
            v65 = [persist.tile([128, 384], MMDT, tag=f"v{t}", name=f"v{t}") for t in range(NT)]
            for nt in range(NT):
                nc.gpsimd.tensor_copy(out=v65[nt][:, 64:128], in_=ones_t[:, 0:64])
                nc.gpsimd.tensor_copy(out=v65[nt][:, 256:320], in_=ones_t[:, 0:64])
                ps = ps_mm.tile([128, CSL], F32, tag="mm")
                for ct in range(8):
                    nc.tensor.matmul(
                        ps,
                        lhsT=xts[ct][:, nt * 128 : (nt + 1) * 128],
                        rhs=wvs[ct],
                        start=(ct == 0),
                        stop=(ct == 7),
                    )
                nc.vector.tensor_copy(out=v65[nt][:, 0:64], in_=ps[:, 0:64])
                nc.vector.tensor_copy(out=v65[nt][:, 128:256], in_=ps[:, 64:192])
                nc.vector.tensor_copy(out=v65[nt][:, 320:384], in_=ps[:, 192:256])

            # stationary windows: (even head A, odd head B) per pair
            def vwin(kt, pair, hh):
                base = pair * 192 + hh * 64
                return v65[kt][:, base : base + 128]

            # ---- phase 3+4: attention (per 512-query block), then out-proj ----
            def emit_outproj(qi, a2):
                # output projection for query block qi's 4 row tiles
                for nt in range(4 * qi, 4 * qi + 4):
                    ysb = y_pool.tile([128, C], F32, tag="y", name=f"ysb{nt}")
                    ntl = (nt - 4 * qi) * 128
                    for cb in range(2):
                        psy = ps_mm.tile([128, QBLK], F32, tag="mm", name="psy")
                        for pr in range(2):
                            nc.tensor.matmul(
                                psy,
                                lhsT=a2[pr][:, ntl : ntl + 128],
                                rhs=wp2[pr][:, cb * QBLK : (cb + 1) * QBLK],
                                start=(pr == 0),
                                stop=(pr == 1),
                            )
                        nc.vector.tensor_copy(
                            out=ysb[:, cb * QBLK : (cb + 1) * QBLK], in_=psy
                        )
                    nc.sync.dma_start(out=y_d[nt * 128 : (nt + 1) * 128, :], in_=ysb)

            pending = None  # (qi, a2) whose out-proj is deferred one block
            for qi in range(NQB):
                a2 = [
                    at_pool.tile([128, QBLK], MMDT, tag=f"a{p}", name=f"a{p}_{qi}")
                    for p in range(2)
                ]
                for pair in range(2):
                    qt = qkT[pair]
                    kt_t = qkT[2 + pair]
                    qs = slice(qi * QBLK, (qi + 1) * QBLK)

                    # one PSUM bank per head: PV rows on one partition half,
                    # replicated key-sums on the other (from the ones block
                    # of the stationary). A: PV 0:64 / sums 64:128; B: sums
                    # 0:64 / PV 64:128.
                    at_bA = ps_acc.tile([128, QBLK], F32, tag="atA", name="at_bA")
                    at_bB = ps_acc.tile([128, QBLK], F32, tag="atB", name="at_bB")

                    n_reg = 4 * qi
                    n_per_range = n_reg + 4
                    at_A, at_B = [0], [0]

                    def fl(cnt, total=n_per_range):
                        i = cnt[0]
                        cnt[0] += 1
                        return dict(start=(i == 0), stop=(i == total - 1))

                    # work items: rect key tiles then diagonal tiles
                    items = [("r", kt) for kt in range(n_reg)]
                    items += [("d", j) for j in range(4)]

                    def emit_scores(item):
                        """score matmuls + exps for one key tile; returns the
                        p tiles + query range for the later PV matmuls."""
                        kind, idx = item
                        psA = ps_sc.tile([128, QBLK], F32, tag="sA", name="psA")
                        psB = ps_sc.tile([128, QBLK], F32, tag="sB", name="psB")
                        if kind == "r":
                            kt, q0 = idx, 0
                            ks = slice(kt * 128, (kt + 1) * 128)
                            nc.tensor.matmul(
                                psA, lhsT=kt_t[0:64, ks], rhs=qt[0:64, qs],
                                start=True, stop=True,
                            )
                            nc.tensor.matmul(
                                psB, lhsT=kt_t[64:128, ks], rhs=qt[64:128, qs],
                                start=True, stop=True,
                            )
                            pA = pt_pool.tile([128, QBLK], MMDT, tag="pA", name="pA")
                            pB = pt_pool.tile([128, QBLK], MMDT, tag="pB", name="pB")
                            nc.scalar.activation(
                                out=pA, in_=psA,
                                func=mybir.ActivationFunctionType.Exp, scale=SCALE,
                            )
                            nc.scalar.activation(
                                out=pB, in_=psB,
                                func=mybir.ActivationFunctionType.Exp, scale=SCALE,
                            )
                        else:
                            j = idx
                            kt = 4 * qi + j
                            q0 = 128 * j
                            q1 = q0 + 64
                            ksl = slice(kt * 128, (kt + 1) * 128)
                            qsl = slice(qi * QBLK + q0, (qi + 1) * QBLK)
                            pA = pd[j, 0]
                            pB = pd[j, 1]
                            for ph, ps_s, p_s in ((0, psA, pA), (64, psB, pB)):
                                hd_sl = slice(ph, ph + 64)
                                nc.tensor.matmul(
                                    ps_s[:, q0:QBLK], lhsT=kt_t[hd_sl, ksl],
                                    rhs=qt[hd_sl, qsl], start=True, stop=True,
                                )
                                # two exps that skirt the pre-zeroed corner
                                nc.scalar.activation(
                                    out=p_s[0:64, q0:QBLK], in_=ps_s[0:64, q0:QBLK],
                                    func=mybir.ActivationFunctionType.Exp,
                                    scale=SCALE,
                                )
                                nc.scalar.activation(
                                    out=p_s[64:128, q1:QBLK],
                                    in_=ps_s[64:128, q1:QBLK],
                                    func=mybir.ActivationFunctionType.Exp,
                                    scale=SCALE,
                                )
                        return kt, q0, pA, pB

                    def emit_pv(staged):
                        kt, q0, pA, pB = staged
                        nc.tensor.matmul(
                            at_bA[:, q0:QBLK], lhsT=vwin(kt, pair, 0),
                            rhs=pA[:, q0:QBLK], **fl(at_A),
                        )
                        nc.tensor.matmul(
                            at_bB[:, q0:QBLK], lhsT=vwin(kt, pair, 1),
                            rhs=pB[:, q0:QBLK], **fl(at_B),
                        )

                    # software-pipelined, lookahead 2: scores run two key
                    # tiles ahead of PV, so the exp latency AND the previous
                    # accumulator bank's normalize-chain latency both hide
                    # under score work; the deferred out-proj of the previous
                    # query block slots in behind the first few tiles.
                    stagedq = [emit_scores(it) for it in items[:2]]
                    for i in range(len(items)):
                        if i + 2 < len(items):
                            stagedq.append(emit_scores(items[i + 2]))
                        if pair == 0 and i == min(2, len(items) - 1) and pending:
                            emit_outproj(*pending)
                            pending = None
                        emit_pv(stagedq.pop(0))

                    # normalize: copy/reciprocal out of PSUM, DMA shifts the
                    # values onto the PV lanes, fast reciprocal for A's half
                    # (custom-DVE ops misread PSUM base 64, so A's sums are
                    # copied out and reciprocated after the shift)
                    rec = rc_pool.tile([128, QBLK], F32, tag="rec")
                    rsh = rc_pool.tile([128, QBLK], F32, tag="rsh")
                    rcf = rc_pool.tile([128, QBLK], F32, tag="rcf")
                    nc.vector.tensor_copy(
                        out=rec[64:128, :], in_=at_bA[64:128, :]
                    )
                    nc.vector.reciprocal_approx_fast(
                        out=rec[0:64, :], in_=at_bB[0:64, :]
                    )
                    nc.sync.dma_start(out=rsh[0:64, :], in_=rec[64:128, :])
                    nc.sync.dma_start(out=rsh[64:128, :], in_=rec[0:64, :])
                    nc.vector.reciprocal_approx_fast(
                        out=rcf[0:64, :], in_=rsh[0:64, :]
                    )
                    nc.vector.tensor_mul(
                        out=a2[pair][0:64, :], in0=at_bA[0:64, :], in1=rcf[0:64, :]
                    )
                    nc.vector.tensor_mul(
                        out=a2[pair][64:128, :], in0=at_bB[64:128, :],
                        in1=rsh[64:128, :],
                    )

                if pending:  # qi=0 has few tiles; flush if not yet emitted
                    emit_outproj(*pending)
                pending = (qi, a2)
            emit_outproj(*pending)

    return nc


def _shard_inputs(x, w_qkv, w_proj):
    import ml_dtypes

    bf16 = ml_dtypes.bfloat16
    x = np.ascontiguousarray(np.asarray(x, dtype=np.float32).astype(bf16))
    w_qkv = np.asarray(w_qkv, dtype=np.float32).astype(bf16)
    w_proj = np.asarray(w_proj, dtype=np.float32).astype(bf16)
    xT = [np.ascontiguousarray(x[b].T).reshape(8, 128, N) for b in range(B)]
    in_maps = []
    for c in range(NCORES):
        b, g = divmod(c, 4)
        r0 = 64 * HPC * g  # 256 * g
        wq = w_qkv[r0 : r0 + CSL, :]
        wk = w_qkv[C + r0 : C + r0 + CSL, :]
        wvs = w_qkv[2 * C + r0 : 2 * C + r0 + CSL, :]
        wqkT = np.ascontiguousarray(np.concatenate([wq, wk], axis=0).T)
        wvT = np.ascontiguousarray(wvs.T)
        wpT = np.ascontiguousarray(w_proj[:, r0 : r0 + CSL].T)
        in_maps.append(
            {
                "xT": xT[b],
                "wqkT": wqkT.reshape(8, 128, QKW),
                "wvT": wvT.reshape(8, 128, CSL),
                "wpT": wpT.reshape(2, 128, C),
            }
        )
    return in_maps


def run(x, w_qkv, w_proj, b_proj, trace=False, **spmd_kwargs):
    from concourse.bass_utils import run_bass_kernel_spmd

    in_maps = _shard_inputs(x, w_qkv, w_proj)
    nc = build_nc()
    nc.finalize()
    res = run_bass_kernel_spmd(
        nc, in_maps, core_ids=list(range(NCORES)), trace=trace, **spmd_kwargs
    )
    y = np.zeros((B, N, C), np.float32)
    for c in range(NCORES):
        y[c // 4] += res.results[c]["y"]
    y += np.asarray(b_proj, dtype=np.float32)[None, None, :]
    return y, res


def kernel(x, w_qkv, w_proj, b_proj):
    y, _ = run(x, w_qkv, w_proj, b_proj, trace=False)
    return y
